# revision 5
# baseline (speedup 1.0000x reference)
"""Trainium2 Bass kernel for nn_Attention (batch=8, seq=1024, dim=1024, 16 heads x 64).

Strategy: pure data parallelism — one batch element per NeuronCore (8 cores),
full weights replicated, zero collectives. Per core:
  LayerNorm (f32 stats) -> qkv matmul in fp32r -> per-head RMS norm (bf16 out)
  -> q/k transposed to [c, tok] via XBAR DMA-transpose (DMA engines, not PE)
  -> scores in bf16 (K=64) -> exp on ScalarE (no max subtraction: |s|<=64<88)
  -> attn@v: stationary = prob chunks, moving = v||ones (bf16), full 128
  output partitions -> batched reciprocal on DVE -> normalize on Pool ->
  DMA-transpose back to [hd, tok] -> out-proj (bf16).
Engine balance: ScalarE keeps only exp + the small ln/exp rsqrt chains (it is
the attention-phase bottleneck), so chunk-2 RMS squares run DVE+Pool, chunk-2
v copies run DVE, and all PSUM->SBUF transpose copies are gone (DMA transpose
writes SBUF directly). gamma (64*gq*gk) is applied to k pre-transpose by Pool
via a partition-replicated gamma tile. Issue order is software-pipelined as
in the earlier PE-transpose version: scores(h+1) before attn@v(h), qkv groups
woven between attention pairs as PE filler.
"""
import sys

sys.path.insert(0, '/opt/trn_rl_repo')

import numpy as np
import ml_dtypes
import concourse.bass as bass
import concourse.mybir as mybir
import concourse.tile as tile
from concourse import bacc
from concourse.bass_utils import run_bass_kernel_spmd

f32 = mybir.dt.float32
f32r = mybir.dt.float32r
bf16 = mybir.dt.bfloat16
AX = mybir.AxisListType
ALU = mybir.AluOpType
ACTF = mybir.ActivationFunctionType

N = 1024          # tokens per core
D = 1024          # model dim
H = 16            # heads
C = 64            # head dim
NT = N // 128     # token tiles
DT = D // 128     # dim tiles

LN_EPS = 1e-5
RMS_EPS = 1e-24


def build():
    nc = bacc.Bacc(None)
    x = nc.declare_dram_parameter("x", [N, D], f32, isOutput=False)
    wqkv = nc.declare_dram_parameter("wqkv", [D, 2 * D], f32r, isOutput=False)
    wv = nc.declare_dram_parameter("wv", [D, D], f32r, isOutput=False)
    wout = nc.declare_dram_parameter("wout", [D, D], bf16, isOutput=False)
    g = nc.declare_dram_parameter("g", [128, D], f32, isOutput=False)
    ident = nc.declare_dram_parameter("ident", [128, 128], f32r, isOutput=False)
    out = nc.declare_dram_parameter("out", [N, D], f32, isOutput=True)

    with tile.TileContext(nc) as tc:
        with tc.tile_pool(name="persist", bufs=1) as pp, \
             tc.tile_pool(name="big", bufs=2) as bigp, \
             tc.tile_pool(name="wstream", bufs=6) as wsp, \
             tc.tile_pool(name="stageA", bufs=2) as sta, \
             tc.tile_pool(name="stageB", bufs=3) as stb, \
             tc.tile_pool(name="kgp", bufs=3) as kgp, \
             tc.tile_pool(name="sqp", bufs=2) as sqp, \
             tc.tile_pool(name="ptpool", bufs=16) as ptp, \
             tc.tile_pool(name="small", bufs=3) as smp, \
             tc.tile_pool(name="osbp", bufs=2) as osbp, \
             tc.tile_pool(name="onpk", bufs=3) as onp, \
             tc.tile_pool(name="osp", bufs=2) as osp, \
             tc.tile_pool(name="ps1024", bufs=2, space="PSUM") as ps1024, \
             tc.tile_pool(name="psb", bufs=2, space="PSUM") as psb, \
             tc.tile_pool(name="tpp", bufs=2, space="PSUM") as tpp:

            # x/out + all XBAR transposes stream on the sync(SP) HWDGE queue;
            # weights stream on the scalar(ACT) queue so neither FIFO stalls
            # the other and no DMA ever blocks the ACT sequencer mid-exp.
            id_sb = pp.tile([128, 128], f32r, tag="ident")
            nc.scalar.dma_start(id_sb[:], ident[:])
            gfull_sb = pp.tile([128, D], f32, tag="gfull")
            nc.scalar.dma_start(gfull_sb[:], g[:])
            def w_dma(grp, quarter):
                w_sb = wsp.tile([128, DT // 4, 512], f32r, tag="wg",
                                name=f"w_{grp}_{quarter}")
                if grp >= 4:
                    src = wv[quarter * 256:(quarter + 1) * 256,
                             (grp - 4) * 512:(grp - 3) * 512]
                else:
                    src = wqkv[quarter * 256:(quarter + 1) * 256,
                               grp * 512:(grp + 1) * 512]
                nc.scalar.dma_start(
                    w_sb[:], src.rearrange("(ko ki) f -> ki ko f", ki=128))
                return w_sb

            w_tiles = {gg: [] for gg in range(6)}
            x_tiles = []
            for tt in range(NT):
                x_sb = sta.tile([128, D], f32, tag="x_t", name=f"x_{tt}")
                nc.sync.dma_start(x_sb[:], x[tt * 128:(tt + 1) * 128, :])
                x_tiles.append(x_sb)
                if tt < 4:
                    w_tiles[4].append(w_dma(4, tt))
            eps_ln = pp.tile([128, 1], f32, tag="epsln")
            nc.gpsimd.memset(eps_ln[:], LN_EPS)
            eps_rms = pp.tile([128, 1], f32, tag="epsrms")
            nc.gpsimd.memset(eps_rms[:], RMS_EPS)
            one_c = pp.tile([128, 1], f32, tag="onec")
            nc.gpsimd.memset(one_c[:], 1.0)
            zz = pp.tile([1, 260], bf16, tag="zz")
            nc.gpsimd.memset(zz[:], 0.0)

            # remaining weights in consumption order; 6-slot ring paces them
            for grp in [0, 2, 5, 1, 3]:
                for quarter in range(4):
                    w_tiles[grp].append(w_dma(grp, quarter))
            wout_q = []
            for q in range(4):
                w_sb = wsp.tile([128, 2, D], bf16, tag="wg", name=f"wo_{q}")
                nc.scalar.dma_start(
                    w_sb[:], wout[q * 256:(q + 1) * 256, :]
                    .rearrange("(ko ki) d -> ki ko d", ki=128))
                wout_q.append(w_sb)

            # Persistent big tensors. xnT / ohn share the 2-slot "big" pool:
            # xnT (slot 0) dies after the last qkv matmul; ohn_a takes slot 1;
            # ohn_b reuses slot 0.
            xnT = bigp.tile([128, DT, N], f32r, tag="big")        # [d, dt, t]
            qnT = pp.tile([128, 4, N], bf16, tag="qnT")           # [2hx64c, pair%4, t]
            knT = pp.tile([128, 4, N], bf16, tag="knT")
            v_aug = pp.tile([128, NT, H, 65], bf16, tag="vaug")   # [j, jt, h, c|1]
            ohn_box = [None, None]
            osb_box = [None, None]
            pts_box = {}

            def ohn_of(p):
                return (ohn_box[0], p) if p < 4 else (ohn_box[1], p - 4)

            # ---------- Phase A: LayerNorm + transpose x ----------
            for tt in range(NT):
                ts = slice(tt * 128, (tt + 1) * 128)
                x_sb = x_tiles[tt]
                s1 = smp.tile([128, 1], f32, tag="s1")
                nc.vector.tensor_reduce(s1[:], x_sb[:], AX.X, ALU.add)
                xn_t = sta.tile([128, D], f32r, tag="xn_t")
                s2 = smp.tile([128, 1], f32, tag="s2")
                # Square output is scratch: write it into xn_t, which the
                # normalize below overwrites anyway (only accum_out is used)
                nc.scalar.activation(xn_t[:, 0:D], x_sb[:], ACTF.Square,
                                     bias=0.0, scale=1.0, accum_out=s2[:])
                m2 = smp.tile([128, 1], f32, tag="m2")
                nc.gpsimd.tensor_tensor(m2[:], s1[:], s1[:], ALU.mult)
                dvar = smp.tile([128, 1], f32, tag="dvar")
                nc.gpsimd.tensor_scalar(dvar[:], m2[:], -1.0 / D, s2[:], ALU.mult, ALU.add)
                lnv = smp.tile([128, 1], f32, tag="lnv")
                nc.scalar.activation(lnv[:], dvar[:], ACTF.Ln, bias=eps_ln[:], scale=1.0 / D)
                rsig = smp.tile([128, 1], f32, tag="rsig")
                nc.scalar.activation(rsig[:], lnv[:], ACTF.Exp, bias=0.0, scale=-0.5)
                nmr = smp.tile([128, 1], f32, tag="nmr")
                nc.gpsimd.tensor_scalar(nmr[:], s1[:], rsig[:], -1.0 / D, ALU.mult, ALU.mult)
                nc.vector.tensor_scalar(xn_t[:, 0:512], x_sb[:, 0:512], rsig[:], nmr[:],
                                        ALU.mult, ALU.add)
                nc.vector.tensor_scalar(xn_t[:, 512:D], x_sb[:, 512:D], rsig[:], nmr[:],
                                        ALU.mult, ALU.add)
                for half in range(2):
                    ps4 = tpp.tile([128, 4, 128], f32r, tag="tp",
                                   name=f"xt_{tt}_{half}")
                    for b in range(4):
                        dt_i = half * 4 + b
                        nc.tensor.transpose(
                            ps4[:, b, :], xn_t[:, dt_i * 128:(dt_i + 1) * 128],
                            id_sb[:])
                    if half == 0:
                        nc.scalar.copy(
                            xnT[:, half * 4:(half + 1) * 4, ts], ps4[:])
                    else:
                        nc.vector.tensor_copy(
                            xnT[:, half * 4:(half + 1) * 4, ts], ps4[:])

            pend = {}

            def group_mm(grp, tt, alt=False):
                """Matmul half of one qkv group token-tile. alt=True borrows
                an idle ps1024 slot for ps_q so the RMS chain never stalls
                the matmul pipeline."""
                w_halves = w_tiles[grp]
                ts = slice(tt * 128, (tt + 1) * 128)
                if alt:
                    ps_q = ps1024.tile([128, 1024], f32, tag="ps1024",
                                       name=f"psq_{grp}_{tt}")[:, 0:512]
                else:
                    ps_q = psb.tile([128, 512], f32, tag="ps512")
                for dt_i in range(DT):
                    nc.tensor.matmul(ps_q[:], xnT[:, dt_i, ts],
                                     w_halves[dt_i // 2][:, dt_i % 2, :],
                                     start=(dt_i == 0), stop=(dt_i == DT - 1))
                return (tt, ps_q)

            def group_fin(grp, st):
                """RMS-norm + XBAR-transpose half (or v copy) of a group tile.
                Early chunk (grp 0/2/4) leans on ACT (idle pre-attention);
                late chunk (grp 1/3/5) keeps ACT free for exp."""
                kind = grp // 2
                chunk = grp % 2
                tt, ps_q = st
                ts = slice(tt * 128, (tt + 1) * 128)
                ps3 = ps_q.rearrange("p (h c) -> p h c", c=64)
                if kind == 2:
                    hbase = chunk * 8
                    if chunk == 0:
                        nc.scalar.copy(
                            v_aug[:, tt, hbase:hbase + 8, 0:64], ps3)
                    else:
                        nc.vector.tensor_copy(
                            v_aug[:, tt, hbase:hbase + 8, 0:64], ps3)
                    return
                ss = smp.tile([128, 8], f32, tag="ss")
                sq = sqp.tile([128, 512], f32, tag="sq")
                sq3 = sq.rearrange("p (h c) -> p h c", c=64)
                nc.scalar.activation(sq3, ps3, ACTF.Square,
                                     bias=0.0, scale=1.0)
                nc.vector.tensor_reduce(ss[:], sq3, AX.X, ALU.add)
                lnss = smp.tile([128, 8], f32, tag="lnss")
                nc.scalar.activation(lnss[:], ss[:], ACTF.Ln, bias=eps_rms[:], scale=1.0)
                rsq = smp.tile([128, 8], f32, tag="rsq")
                nc.scalar.activation(rsq[:], lnss[:], ACTF.Exp, bias=0.0, scale=-0.5)
                qn_t = stb.tile([128, 512], bf16, tag="qn_t")
                nc.vector.tensor_tensor(
                    qn_t.rearrange("p (h c) -> p h c", c=64), ps3,
                    rsq[:, :, None].to_broadcast((128, 8, 64)), ALU.mult)
                if kind == 1:
                    kg = kgp.tile([128, 512], bf16, tag="kg")
                    nc.gpsimd.tensor_tensor(
                        kg[:], qn_t[:],
                        gfull_sb[:, chunk * 512:(chunk + 1) * 512], ALU.mult)
                    nc.sync.dma_start_transpose(knT[:, :, ts], kg[:])
                else:
                    nc.sync.dma_start_transpose(qnT[:, :, ts], qn_t[:])

            def group_tt(grp, tt, alt=False):
                """Software-pipelined group step: the finish half (RMS chain +
                transpose DMA) runs one token-tile behind the matmuls, so the
                PE never waits on the chain."""
                if grp in pend:
                    group_fin(grp, pend.pop(grp))
                pend[grp] = group_mm(grp, tt, alt)

            def group_flush(grp):
                if grp in pend:
                    group_fin(grp, pend.pop(grp))

            def scores_jts(h, jt_lo, jt_hi):
                p = h // 2
                pc = p % 4
                hp = slice((h % 2) * 64, (h % 2) * 64 + 64)
                if h not in pts_box:
                    pts_box[h] = [
                        ptp.tile([128, N], bf16, tag="pT", name=f"pT_{h}_{jt}")
                        for jt in range(NT)]
                pts = pts_box[h]
                for jt in range(jt_lo, jt_hi):
                    ps_s = ps1024.tile([128, 1024], f32, tag="ps1024")
                    for ih in range(2):
                        nc.tensor.matmul(
                            ps_s[:, ih * 512:(ih + 1) * 512],
                            knT[hp, pc, jt * 128:(jt + 1) * 128],
                            qnT[hp, pc, ih * 512:(ih + 1) * 512],
                            start=True, stop=True)
                    nc.scalar.activation(pts[jt][:], ps_s[:],
                                         ACTF.Exp, bias=0.0, scale=1.0)

            def scores_head(h):
                scores_jts(h, 0, NT)

            def attnv_half(h, half, pts):
                # attn@v flipped: out[i, d(+denom)] — stationary = prob chunk,
                # moving = v||1. 4 sub-bank accumulation groups per PSUM tile.
                ch = h // 8
                hl = h - 8 * ch
                osb_c = osb_box[ch]
                ps4 = tpp.tile([128, 4, 128], f32, tag="tp",
                               name=f"pso_{h}_{half}")
                # A multi-matmul accumulation group must own its PSUM bank,
                # so we cannot run 4 start..stop groups in this one-bank
                # tile. Instead every matmul is a self-contained group:
                # the first per range writes (start=True zeroes only its
                # own output range), the rest accumulate-write.
                for k in range(4):
                    it = half * 4 + k
                    for jt in range(NT):
                        nc.tensor.matmul(
                            ps4[0:128, k, 0:65],
                            pts[jt][:, it * 128:(it + 1) * 128],
                            v_aug[:, jt, h, 0:65],
                            start=(jt == 0), stop=True,
                            skip_group_check=True)
                nc.vector.tensor_copy(
                    osb_c[:, half * 4:(half + 1) * 4, hl, 0:65],
                    ps4[:, :, 0:65])

            def attnv_head(h):
                pts = pts_box.pop(h)
                attnv_half(h, 0, pts)
                attnv_half(h, 1, pts)

            def new_chunk(ch):
                osb_box[ch] = osbp.tile([128, NT, 8, 65], bf16, tag="osb",
                                        name=f"osb_{ch}")

            def finish_its(ch, its_range):
                """Batched reciprocal (DVE) + normalize (Pool) + XBAR
                transpose to ohn."""
                ohn = ohn_box[ch]
                osb_c = osb_box[ch]
                for it in its_range:
                    its = slice(it * 128, (it + 1) * 128)
                    den = smp.tile([128, 8], f32, tag="den", name=f"den_{ch}_{it}")
                    nc.gpsimd.tensor_copy(
                        den[:], osb_c[:, it, :, 64:65].rearrange("p h o -> p (h o)"))
                    r_f = smp.tile([128, 8], f32, tag="rf", name=f"rf_{ch}_{it}")
                    nc.vector.reciprocal(r_f[:], den[:])
                    on_pk = onp.tile([128, 512], bf16, tag="onpk",
                                     name=f"onpk_{ch}_{it}")
                    nc.gpsimd.tensor_tensor(
                        on_pk.rearrange("p (h c) -> p h c", c=64),
                        osb_c[:, it, :, 0:64],
                        r_f[:, :, None].to_broadcast((128, 8, 64)), ALU.mult)
                    nc.sync.dma_start_transpose(ohn[:, :, its], on_pk[:])

            def proj_it(it):
                its = slice(it * 128, (it + 1) * 128)
                for dh in range(2):
                    if dh == 0:
                        ps_f = psb.tile([128, 512], f32, tag="ps512")
                    else:
                        ps_f = ps1024.tile([128, 1024], f32, tag="ps1024",
                                           name=f"psf_{it}_{dh}")[:, 0:512]
                    for p in range(8):
                        ohn, po = ohn_of(p)
                        nc.tensor.matmul(
                            ps_f[:], ohn[:, po, its],
                            wout_q[p // 2][:, p % 2, dh * 512:(dh + 1) * 512],
                            start=(p == 0), stop=(p == 7))
                    o_sb = osp.tile([128, 512], f32, tag="o_sb",
                                    name=f"o_sb_{it}_{dh}")
                    nc.vector.tensor_copy(o_sb[:], ps_f[:])
                    nc.sync.dma_start(out[its, dh * 512:(dh + 1) * 512], o_sb[:])

            # ---------- Phases B/C/D: pipelined attention + woven qkv ----------
            # Lifetime constraints: qnT/knT are single-buffered, so group 1
            # (q ch2) must issue after ALL chunk-1 scores; group 3 (k ch2) is
            # consumed jt-granularly by chunk-2 scores and weaves with them.
            for tt in range(NT):
                group_tt(4, tt, alt=(tt % 2 == 1))   # v heads 0..7
            group_flush(4)
            nc.gpsimd.memset(v_aug[:, :, 0:8, 64:65], 1.0)
            for tt in range(NT):
                group_tt(0, tt, alt=(tt % 2 == 1))   # q heads 0..7
            group_flush(0)
            ohn_box[0] = bigp.tile([128, 4, N], bf16, tag="big", name="ohn_a")
            new_chunk(0)
            # k heads 0..7 with S0 woven in jt-granularly (2-tt safety lag)
            group_tt(2, 0)
            group_tt(2, 1)
            for i in range(NT - 2):
                group_tt(2, i + 2)
                scores_jts(0, i, i + 1)
            group_flush(2)
            scores_jts(0, NT - 2, NT)
            # chunk 1: S(h+1) before A(h); g5 (v ch2) spread uniformly
            scores_head(1)
            attnv_head(0)
            group_tt(5, 0)
            scores_head(2)
            attnv_head(1)
            group_tt(5, 1)
            scores_head(3)
            attnv_head(2)
            group_tt(5, 2)
            scores_head(4)
            attnv_head(3)
            group_tt(5, 3)
            scores_head(5)
            attnv_head(4)
            group_tt(5, 4)
            scores_head(6)
            attnv_head(5)
            group_tt(5, 5)
            scores_head(7)
            attnv_head(6)
            group_tt(5, 6)
            group_tt(5, 7)
            group_flush(5)
            nc.gpsimd.memset(v_aug[:, :, 8:16, 64:65], 1.0)
            for tt in range(4):
                group_tt(1, tt, alt=(tt % 2 == 1))  # q ch2: after chunk-1 scores
            attnv_head(7)
            for tt in range(4, NT):
                group_tt(1, tt, alt=(tt % 2 == 1))
            group_flush(1)

            ohn_box[1] = bigp.tile([128, 4, N], bf16, tag="big", name="ohn_b")
            new_chunk(1)

            # chunk 2: g3 (k ch2) weaves jt-granularly into S8;
            # finish_chunk(0) spread uniformly as PE filler
            group_tt(3, 0)
            group_tt(3, 1)
            for i in range(NT - 2):
                group_tt(3, i + 2)
                scores_jts(8, i, i + 1)
            group_flush(3)
            scores_jts(8, NT - 2, NT)
            scores_head(9)
            attnv_head(8)
            finish_its(0, range(0, 1))
            scores_head(10)
            attnv_head(9)
            finish_its(0, range(1, 2))
            scores_head(11)
            attnv_head(10)
            finish_its(0, range(2, 3))
            scores_head(12)
            attnv_head(11)
            finish_its(0, range(3, 4))
            scores_head(13)
            attnv_head(12)
            finish_its(0, range(4, 5))
            scores_head(14)
            attnv_head(13)
            finish_its(0, range(5, 6))
            scores_head(15)
            attnv_head(14)
            finish_its(0, range(6, NT))
            attnv_head(15)
            # tail pipeline: finish chunk-2 per-it, proj follows 2 its behind
            finish_its(1, range(0, 2))
            finish_its(1, range(2, 4))
            proj_it(0)
            finish_its(1, range(4, 6))
            proj_it(1)
            proj_it(2)
            finish_its(1, range(6, NT))
            for it in range(3, NT):
                proj_it(it)
    return nc


_NC_CACHE = None


def _patch_act_tables():
    """Steer bacc's greedy act-table-set selection to natural_log_exp_and_others
    for every function this kernel uses (exp/ln/square/copy/identity), by
    hiding those functions from all earlier sets. Set order (and thus the
    act_func_set_id each load emits) is unchanged, so the runtime tables are
    correct — but all our activations resolve to one co-resident set and the
    kernel performs a single table load instead of thrashing."""
    import collections
    import concourse.bacc as _bacc
    import concourse.hw_specs as _hw
    orig = getattr(_hw.get_activation_tables, '__wrapped_orig__', _hw.get_activation_tables)

    def patched(arch):
        d = orig(arch)
        key = "natural_log_exp_and_others"
        if key not in d:
            return d
        mine = d[key]
        hidden = {f for f in mine}
        nd = collections.OrderedDict()
        for k, v in d.items():
            if k == key:
                nd[k] = v
            else:
                nd[k] = v - hidden
        return nd
    patched.__wrapped_orig__ = orig
    _hw.get_activation_tables = patched
    _bacc.get_activation_tables = patched


def _get_nc():
    global _NC_CACHE
    if _NC_CACHE is None:
        _patch_act_tables()
        nc = build()
        nc.finalize()
        _NC_CACHE = nc
    return _NC_CACHE


def kernel(x, ln_gamma, q_gamma, k_gamma, w_qkv, w_out):
    x = np.asarray(x, dtype=np.float32)
    ln_gamma = np.asarray(ln_gamma, dtype=np.float32)
    q_gamma = np.asarray(q_gamma, dtype=np.float32).reshape(H, C)
    k_gamma = np.asarray(k_gamma, dtype=np.float32).reshape(H, C)
    w_qkv = np.asarray(w_qkv, dtype=np.float32)
    w_out = np.asarray(w_out, dtype=np.float32)

    wqkv_eff = ln_gamma[:, None] * w_qkv
    wqk = np.ascontiguousarray(wqkv_eff[:, 0:2048], dtype=np.float32)
    wv_bf = np.ascontiguousarray(wqkv_eff[:, 2048:], dtype=np.float32)
    wout_bf = w_out.astype(ml_dtypes.bfloat16)
    gfull = np.tile((64.0 * q_gamma * k_gamma).reshape(1, H * C),
                    (128, 1)).astype(np.float32)
    ident = np.eye(128, dtype=np.float32)

    nc = _get_nc()
    in_maps = [
        {"x": np.ascontiguousarray(x[i]), "wqkv": wqk, "wv": wv_bf,
         "wout": wout_bf, "g": gfull, "ident": ident}
        for i in range(8)
    ]
    res = run_bass_kernel_spmd(nc, in_maps, core_ids=list(range(8)))
    return np.stack([res.results[i]["out"] for i in range(8)], axis=0)
